# revision 34
# baseline (speedup 1.0000x reference)
"""Chamfer deviation L2 kernel for Trainium2 (8 NeuronCores, data-parallel over batch).

Contract: kernel(xyz1, xyz2) takes FULL inputs [8, 4096, 3] fp32 and returns the
FULL output [4] fp32 (cd_l2 scalar + 3-vector mean deviation).

Algorithm (per core = one batch b):
  x[n, m] = -d[n, m] = 2*x1.x2 - |x1|^2 - |x2|^2 computed on the PE via a
  5-term augmented contraction in fp32 PSUM. Production is 2-way row-packed:
  two K=5 matmuls at array row-groups 0 and 32 share one streamed operand and
  fill the two 512-col halves of a [128, 1024] PSUM tile concurrently.
  Two orientations:
    A: tiles [n_part=128, m_free]   (weights = xyz1 tiles, stream = xyz2)
    B: tiles [m_part=128, n_free]   (weights = xyz2 tiles, stream = xyz1)
  Per orientation (direction):
    detect: DVE row-max over the free dim -> ng[p] = -max = dmin (dist values)
    mask:   recompute x (bitwise-identical packed matmuls), ScalarE
            Sign(x + ng) via per-partition bias: 0 exactly at argmin, -1 else
    count:  PE bf16 ones-column matmuls sum the sign tile over partitions,
            accumulated in PSUM per quarter; partial counts DMA'd out.
  Host: dist sums from ng1/ng2; c1[m] = #times xyz2[m] is a nearest neighbour
  => V1 = sum_m c1[m]*xyz2[m]; final [4] assembled in float64, cast to fp32.
"""

import os
import sys

sys.path.insert(0, "/opt/trn_rl_repo")

import numpy as np

import concourse.bass as bass
import concourse.bacc as bacc
import concourse.tile as tile
from concourse import mybir
from concourse.bass_utils import run_bass_kernel_spmd

F32 = mybir.dt.float32
BF16 = mybir.dt.bfloat16
AX = mybir.AxisListType
OP = mybir.AluOpType
AF = mybir.ActivationFunctionType

B, N, M = 8, 4096, 4096
P = 128
QT = 8    # weight tiles per quarter
CH = 1024  # psum work-tile width (2 packed 512-col halves)


def build_nc(n=N, m=M, loop=1, packed=True, f32r=False):
    """Build the per-core Bacc program (SPMD: same program on all 8 cores).
    loop > 1 repeats the whole compute body (for wall-clock benchmarking)."""
    assert n == m and n % CH == 0
    nt = n // P             # weight tiles per orientation (32)
    nch = n // CH           # 1024-wide stream chunks (4)
    nq = (nt + QT - 1) // QT
    MMDT = mybir.dt.float32r if f32r else F32

    nc = bacc.Bacc("TRN2", target_bir_lowering=False, debug=False)

    # weights, replicated at partition rows 0-4 and 32-36
    d_wA = nc.dram_tensor("wrepA", [64, n], MMDT, kind="ExternalInput")
    d_wB = nc.dram_tensor("wrepB", [64, n], MMDT, kind="ExternalInput")
    # packed streams: col-block j holds chunks 2j (parts 0-4) and 2j+1 (parts 32-36)
    sshape = [64, n // 2] if packed else [5, n]
    d_sA = nc.dram_tensor("strmA", sshape, MMDT, kind="ExternalInput")
    d_sB = nc.dram_tensor("strmB", sshape, MMDT, kind="ExternalInput")
    d_ones = nc.dram_tensor("onescol", [P, 1], BF16, kind="ExternalInput")

    d_ng1 = nc.dram_tensor("ng1", [P, nt], F32, kind="ExternalOutput")
    d_ng2 = nc.dram_tensor("ng2", [P, nt], F32, kind="ExternalOutput")
    d_c1p = nc.dram_tensor("c1p", [nq, m], F32, kind="ExternalOutput")
    d_c2p = nc.dram_tensor("c2p", [nq, n], F32, kind="ExternalOutput")

    with tile.TileContext(nc) as tc:
        from contextlib import ExitStack

        with ExitStack() as ctx:
            cpool = ctx.enter_context(tc.tile_pool(name="const", bufs=1))
            work_ps = ctx.enter_context(
                tc.tile_pool(name="workps", bufs=3, space="PSUM")
            )
            acc_ps = ctx.enter_context(tc.tile_pool(name="accps", bufs=1, space="PSUM"))
            scr_pool = ctx.enter_context(tc.tile_pool(name="scr", bufs=3))

            wA = cpool.tile([64, n], MMDT, tag="wA")
            wB = cpool.tile([64, n], MMDT, tag="wB")
            sA = cpool.tile(sshape, MMDT, tag="sA")
            sB = cpool.tile(sshape, MMDT, tag="sB")
            onescol = cpool.tile([P, 1], BF16, tag="onescol")
            ng1_sb = cpool.tile([P, nt], F32, tag="ng1sb")
            ng2_sb = cpool.tile([P, nt], F32, tag="ng2sb")
            partA = cpool.tile([P, nt * nch], F32, tag="partA")
            partB = cpool.tile([P, nt * nch], F32, tag="partB")

            nc.sync.dma_start(wA[:, :], d_wA.ap())
            nc.sync.dma_start(wB[:, :], d_wB.ap())
            nc.sync.dma_start(sA[:, :], d_sA.ap())
            nc.sync.dma_start(sB[:, :], d_sB.ap())
            nc.sync.dma_start(onescol[:, :], d_ones.ap())

            def x_chunk(pt, w, s, t, c):
                """Produce x for weight-tile t, columns [c*CH, (c+1)*CH) into pt.
                Two row-packed K=5 matmuls share the streamed block c."""
                blk = s[:, c * 512:(c + 1) * 512]
                for g in range(2):
                    nc.tensor.matmul(
                        pt[:, g * 512:(g + 1) * 512],
                        lhsT=w[32 * g:32 * g + 5, t * P:(t + 1) * P],
                        rhs=blk[32 * g:32 * g + 5, :],
                        start=True,
                        stop=True,
                        tile_position=(32 * g, 0),
                    )

            def x_chunk_flat(pt, w, s, t, c):
                """Unpacked fallback: two sequential K=5 matmuls from row 0."""
                for h in range(2):
                    nc.tensor.matmul(
                        pt[:, h * 512:(h + 1) * 512],
                        lhsT=w[0:5, t * P:(t + 1) * P],
                        rhs=s[0:5, c * CH + h * 512:c * CH + (h + 1) * 512],
                        start=True,
                        stop=True,
                    )

            produce = x_chunk if packed else x_chunk_flat

            def det_tile(w, s, part, ng_sb, t):
                for c in range(nch):
                    pt = work_ps.tile([P, CH], F32, tag="workps")
                    produce(pt, w, s, t, c)
                    nc.vector.tensor_reduce(
                        part[:, t * nch + c:t * nch + c + 1],
                        pt[:, :],
                        axis=AX.X,
                        op=OP.max,
                    )
                nc.vector.tensor_reduce(
                    ng_sb[:, t:t + 1],
                    part[:, t * nch:(t + 1) * nch],
                    axis=AX.X,
                    op=OP.max,
                    negate=True,
                )

            def mask_quarter(w, s, ng_sb, d_cp, q):
                tlist = list(range(q * QT, min((q + 1) * QT, nt)))
                for c in range(nch):
                    acc = acc_ps.tile([1, CH], F32, tag="accps")
                    for ti, t in enumerate(tlist):
                        pm = work_ps.tile([P, CH], F32, tag="workps")
                        produce(pm, w, s, t, c)
                        scr = scr_pool.tile([P, CH], BF16)
                        nc.scalar.activation(
                            scr[:, :],
                            pm[:, :],
                            AF.Sign,
                            bias=ng_sb[:, t:t + 1],
                            scale=1.0,
                        )
                        for h in range(2):
                            nc.tensor.matmul(
                                acc[0:1, h * 512:(h + 1) * 512],
                                lhsT=onescol[:, 0:1],
                                rhs=scr[:, h * 512:(h + 1) * 512],
                                start=(ti == 0),
                                stop=(ti == len(tlist) - 1),
                            )
                    sacc = scr_pool.tile([1, CH], F32, tag="sacc", name="sacc")
                    nc.scalar.copy(sacc[0:1, :], acc[0:1, :])
                    nc.sync.dma_start(
                        d_cp.ap()[q:q + 1, c * CH:(c + 1) * CH], sacc[0:1, :]
                    )

            def body():
                for q in range(nq):
                    for t in range(q * QT, min((q + 1) * QT, nt)):
                        det_tile(wA, sA, partA, ng1_sb, t)
                    mask_quarter(wA, sA, ng1_sb, d_c1p, q)
                for q in range(nq):
                    for t in range(q * QT, min((q + 1) * QT, nt)):
                        det_tile(wB, sB, partB, ng2_sb, t)
                    mask_quarter(wB, sB, ng2_sb, d_c2p, q)

                nc.sync.dma_start(d_ng1.ap(), ng1_sb[:, :])
                nc.sync.dma_start(d_ng2.ap(), ng2_sb[:, :])

            if loop > 1:
                with tc.For_i(0, loop, 1):
                    body()
            else:
                body()

    nc.compile()
    return nc


def _augment(xyz, n):
    """[n,3] -> (lhs_aug [5,n] weights-side, rhs_aug [5,n] stream-side).
    Term order is fixed: 2xx, 2yy, 2zz, -sq_w, -sq_s."""
    x, y, z = xyz[:, 0].copy(), xyz[:, 1].copy(), xyz[:, 2].copy()
    sq = (x * x + y * y) + z * z
    one = np.ones(n, np.float32)
    lhs = np.stack([2 * x, 2 * y, 2 * z, -sq, -one]).astype(np.float32)
    rhs = np.stack([x, y, z, one, sq]).astype(np.float32)
    return lhs, rhs


def make_inputs(xyz1b, xyz2b, n=N, m=M, packed=True):
    """Build packed augmented operands for one batch."""
    import ml_dtypes

    assert n == m
    lhs1, rhs1 = _augment(xyz1b, n)   # weights-A / stream-B aug (xyz1)
    lhs2, rhs2 = _augment(xyz2b, m)   # weights-B / stream-A aug (xyz2)

    def wrep(lhs):
        w = np.zeros((64, n), np.float32)
        w[0:5] = lhs
        w[32:37] = lhs
        return w

    def spack(rhs):
        if not packed:
            return rhs
        s = np.zeros((64, n // 2), np.float32)
        r3 = rhs.reshape(5, n // 1024, 2, 512)
        s3 = s.reshape(64, n // 1024, 512)
        s3[0:5] = r3[:, :, 0, :]
        s3[32:37] = r3[:, :, 1, :]
        return s

    return {
        "wrepA": wrep(lhs1),
        "strmA": spack(rhs2),
        "wrepB": wrep(lhs2),
        "strmB": spack(rhs1),
        "onescol": np.ones((P, 1), ml_dtypes.bfloat16),
    }


def decode_core(out, xyz1b, xyz2b, n=N, m=M, verbose=False):
    """Decode one core's outputs into partial sums (float64)."""
    ng1 = out["ng1"].astype(np.float64)   # [128, nt]; dist1, n = t*128 + p
    ng2 = out["ng2"].astype(np.float64)
    dist1 = ng1.T.reshape(n)
    dist2 = ng2.T.reshape(m)

    # colsum of sign tiles: sum over quarters = c - n_total (sign(0)=0)
    c1 = out["c1p"].astype(np.float64).sum(axis=0) + n
    c2 = out["c2p"].astype(np.float64).sum(axis=0) + m
    if verbose:
        print(
            f"  count sums: c1={c1.sum():.1f} (want {n}), c2={c2.sum():.1f} (want {m})"
        )

    V1 = c1 @ xyz2b.astype(np.float64)
    V2 = c2 @ xyz1b.astype(np.float64)
    return dist1.sum(), dist2.sum(), V1, V2, c1.sum(), c2.sum()


_NC_CACHE = {}
LAST_RESULTS = None


def kernel(xyz1, xyz2, trace=False, verbose=False):
    global LAST_RESULTS
    xyz1 = np.asarray(xyz1, dtype=np.float32)
    xyz2 = np.asarray(xyz2, dtype=np.float32)
    b, n, _ = xyz1.shape
    m = xyz2.shape[1]

    key = (n, m)
    if key not in _NC_CACHE:
        _NC_CACHE[key] = build_nc(n, m)
    nc = _NC_CACHE[key]

    in_maps = [make_inputs(xyz1[i], xyz2[i], n, m) for i in range(b)]
    res = run_bass_kernel_spmd(
        nc, in_maps, core_ids=list(range(b)), trace=trace
    )
    LAST_RESULTS = res

    S1 = S2 = 0.0
    V1 = np.zeros(3)
    V2 = np.zeros(3)
    for i in range(b):
        s1, s2, v1, v2, c1s, c2s = decode_core(
            res.results[i], xyz1[i], xyz2[i], n, m, verbose=verbose
        )
        if abs(c1s - n) > 0.5 or abs(c2s - m) > 0.5:
            print(
                f"kernel: warning core {i}: count sums c1={c1s:.1f}/{n} "
                f"c2={c2s:.1f}/{m}"
            )
        S1 += s1
        S2 += s2
        V1 += v1
        V2 += v2

    sum1 = xyz1.astype(np.float64).sum(axis=(0, 1))
    sum2 = xyz2.astype(np.float64).sum(axis=(0, 1))
    cd_l2 = S1 / (b * n) + S2 / (b * m)
    cd_dev = (sum1 - V1) / (b * n) + (sum2 - V2) / (b * m)
    return np.concatenate([[cd_l2], cd_dev]).astype(np.float32)
